# revision 14
# baseline (speedup 1.0000x reference)
"""Farthest Point Sampling (FPS) on 8 TRN2 NeuronCores.

Problem: x [32, 65536, 6] f32; per batch, iteratively select NPOINT=1024
indices: far_0 = 0; repeat: d = min(d, |p - p_far|^2 (xyz only)),
far_{k+1} = argmax(d). Output [32, 1024] int32 of the selected indices.

Sharding: pure data parallel — 4 batches per core (32 / 8 cores).

Per-core kernel design (all arithmetic bit-exact vs the jax reference:
ACT-square with single-rounding FMA bias == separate sub+mul; all PE
matmuls only multiply by exact 0.0/1.0 values):

  Layout: per "group" of 2 batches, coord tiles X/Y/Z [128, 2*512]
  (point i of batch j lives at partition i//512, column j*512 + i%512),
  distance D [128, 2*512].

  Per iteration (per group, the two groups' chains are independent and
  pipeline against each other):
    ACT : sq_c = Square(-1 * C + cx)  for c in x,y,z, j in 0,1  (6 ops)
    DVE : s = sqx + sqy; s2 = s + sqz; D = min(D, s2)
    DVE : rowmax [128,2] = reduce_max_X(D as [128,2,512])
    PE  : transpose rowmax -> [2,128]; ACT copy -> SBUF
    DVE : max8 -> gmax8 [2,8]; max_index -> p* [2,8]u32 (first occurrence)
    DVE : p*F cast; PE transpose -> [1,2]; ACT copy; PE ones-bcast -> [128,2]
    DVE : rowmask = (piota == p*bcast); msk_j = rowmask * colsel_j
    PE  : 2 accumulating one-hot matmuls -> drows [2,512] (winner rows of D)
    ACT : copy drows -> SBUF
    DVE : max_index(gmax8, drows) -> q* [2,8]u32
    DVE : gloc = p**512 + q* (the output index); gflat = gloc + b*65536
    PE  : transpose gloc/gflat -> rows; ACT copies; PE ones-bcast gflat
    DVE : cast -> u32 gather offsets [128,2]; write gloc -> out row
    GPSIMD: 2 indirect gathers xyz_table[gflat] -> bias [128,3] per batch

Host side only reshapes/shards inputs and reassembles the output.
"""

import sys

if "/opt/trn_rl_repo" not in sys.path:
    sys.path.insert(0, "/opt/trn_rl_repo")

import numpy as np

import concourse.bass as bass
import concourse.mybir as mybir
import concourse.tile as tile
from concourse import bacc
from concourse.bass_utils import run_bass_kernel_spmd

P = 128
W = 512  # columns per batch
NB = 4  # batches per core
NG = 2  # groups per core
GB = 2  # batches per group
N = P * W  # 65536 points per batch
NPOINT = 1024
NITER = NPOINT - 1  # 1023 argmax iterations (far_0 = 0 is free)

f32 = mybir.dt.float32
u32 = mybir.dt.uint32
i32 = mybir.dt.int32

Act = mybir.ActivationFunctionType
Alu = mybir.AluOpType
Ax = mybir.AxisListType

_CACHED = {}


def build_nc(niter=NITER):
    nc = bacc.Bacc("TRN2", target_bir_lowering=False, debug=False)

    xin = nc.dram_tensor("x", [NB, P, W * 6], f32, kind="ExternalInput")
    identin = nc.dram_tensor("ident", [P, P], f32, kind="ExternalInput")
    onesin = nc.dram_tensor("ones1", [1, P], f32, kind="ExternalInput")
    piotain = nc.dram_tensor("piota", [P, GB], f32, kind="ExternalInput")
    colselin = nc.dram_tensor("colsel", [P, GB * GB], f32, kind="ExternalInput")
    bbasefin = nc.dram_tensor("bbasef", [GB, NG], f32, kind="ExternalInput")
    binitin = nc.dram_tensor("binit", [GB, NG], u32, kind="ExternalInput")
    piotaBigin = nc.dram_tensor("piotaBig", [GB, P], f32, kind="ExternalInput")
    qiotaBigin = nc.dram_tensor("qiotaBig", [GB, W], f32, kind="ExternalInput")
    negbigin = nc.dram_tensor("negbig", [GB, 1], f32, kind="ExternalInput")
    iota1kin = nc.dram_tensor("iota1k", [GB, NPOINT], f32, kind="ExternalInput")
    iotaBig1kin = nc.dram_tensor("iotaBig1k", [GB, NPOINT], f32, kind="ExternalInput")

    out = nc.dram_tensor("out", [NB, NPOINT], i32, kind="ExternalOutput")
    # xyz gather table: row (b*65536 + i) = (x, y, z) of point i of batch b
    tbl = nc.dram_tensor("tbl", [NB * N, 3], f32)

    with tile.TileContext(nc) as tc:
        with (
            tc.tile_pool(name="sb", bufs=1) as sb,
            tc.tile_pool(name="ps", bufs=1, space="PSUM") as ps,
        ):
            ident = sb.tile([P, P], f32, tag="ident", name="ident")
            ones1 = sb.tile([1, P], f32, tag="ones1", name="ones1")
            piota = sb.tile([P, GB], f32, tag="piota", name="piota")
            colsel = sb.tile([P, GB * GB], f32, tag="colsel", name="colsel")
            bbasef = sb.tile([GB, NG], f32, tag="bbasef", name="bbasef")
            binit = sb.tile([GB, NG], u32, tag="binit", name="binit")
            piotaBig = sb.tile([GB, P], f32, tag="piotaBig", name="piotaBig")
            qiotaBig = sb.tile([GB, W], f32, tag="qiotaBig", name="qiotaBig")
            negbig = sb.tile([GB, 1], f32, tag="negbig", name="negbig")
            iota1k = sb.tile([GB, NPOINT], f32, tag="iota1k", name="iota1k")
            iotaBig1k = sb.tile([GB, NPOINT], f32, tag="iotaBig1k", name="iotaBig1k")
            nc.sync.dma_start(ident[:], identin[:])
            nc.sync.dma_start(ones1[:], onesin[:])
            nc.sync.dma_start(piota[:], piotain[:])
            nc.sync.dma_start(colsel[:], colselin[:])
            nc.sync.dma_start(bbasef[:], bbasefin[:])
            nc.sync.dma_start(binit[:], binitin[:])
            nc.sync.dma_start(piotaBig[:], piotaBigin[:])
            nc.sync.dma_start(qiotaBig[:], qiotaBigin[:])
            nc.sync.dma_start(negbig[:], negbigin[:])
            nc.sync.dma_start(iota1k[:], iota1kin[:])
            nc.sync.dma_start(iotaBig1k[:], iotaBig1kin[:])

            # ---- load + repack inputs ----
            C = [[sb.tile([P, GB * W], f32, tag=f"c{c}g{g}", name=f"c{c}g{g}") for c in range(3)]
                 for g in range(NG)]
            # strided one-time DMAs: coords in, gather table out
            tbl_c = tbl.ap().rearrange("(b p q) c -> b p q c", b=NB, p=P)
            for g in range(NG):
                for j in range(GB):
                    b = g * GB + j
                    x6 = xin[b].rearrange("p (q c) -> p q c", c=6)
                    for c in range(3):
                        for h in range(4):  # chunk partitions: 16-bit AP fields
                            pp = slice(h * 32, (h + 1) * 32)
                            nc.sync.dma_start(
                                C[g][c][pp, j * W : (j + 1) * W], x6[pp, :, c]
                            )
                            nc.sync.dma_start(
                                tbl_c[b, pp, :, c],
                                C[g][c][pp, j * W : (j + 1) * W],
                            )

            # ---- state ----
            D = [sb.tile([P, GB * W], f32, tag=f"D{g}", name=f"D{g}") for g in range(NG)]
            S1 = [sb.tile([P, GB * W], f32, tag=f"S1{g}", name=f"S1{g}") for g in range(NG)]
            S2 = [sb.tile([P, GB * W], f32, tag=f"S2{g}", name=f"S2{g}") for g in range(NG)]
            SQ = [[sb.tile([P, GB * W], f32, tag=f"sq{c}g{g}", name=f"sq{c}g{g}") for c in range(3)]
                  for g in range(NG)]
            bias = [sb.tile([P, GB * 3], f32, tag=f"bias{g}", name=f"bias{g}") for g in range(NG)]
            rowmax = [sb.tile([P, GB], f32, tag=f"rmax{g}", name=f"rmax{g}") for g in range(NG)]
            rmT = [sb.tile([GB, P], f32, tag=f"rmT{g}", name=f"rmT{g}") for g in range(NG)]
            gmaxF = [sb.tile([GB, 1], f32, tag=f"gmaxF{g}", name=f"gmaxF{g}") for g in range(NG)]
            eqn = [sb.tile([GB, P], f32, tag=f"eqn{g}", name=f"eqn{g}") for g in range(NG)]
            cand = [sb.tile([GB, P], f32, tag=f"cand{g}", name=f"cand{g}") for g in range(NG)]
            eqn2 = [sb.tile([GB, W], f32, tag=f"eqn2{g}", name=f"eqn2{g}") for g in range(NG)]
            cand2 = [sb.tile([GB, W], f32, tag=f"cand2{g}", name=f"cand2{g}") for g in range(NG)]
            pF = [sb.tile([GB, 1], f32, tag=f"pF{g}", name=f"pF{g}") for g in range(NG)]
            qF = [sb.tile([GB, 1], f32, tag=f"qF{g}", name=f"qF{g}") for g in range(NG)]
            gloc = [sb.tile([GB, 1], f32, tag=f"gloc{g}", name=f"gloc{g}") for g in range(NG)]
            gflat = [sb.tile([GB, 1], f32, tag=f"gflat{g}", name=f"gflat{g}") for g in range(NG)]
            goff = [sb.tile([GB, 1], u32, tag=f"goff{g}", name=f"goff{g}") for g in range(NG)]
            prow = [sb.tile([1, GB], f32, tag=f"prow{g}", name=f"prow{g}") for g in range(NG)]
            pbc_sb = [sb.tile([P, GB], f32, tag=f"pbcs{g}", name=f"pbcs{g}") for g in range(NG)]
            rowmask = [sb.tile([P, GB], f32, tag=f"rmask{g}", name=f"rmask{g}") for g in range(NG)]
            msk = [[sb.tile([P, GB], f32, tag=f"msk{g}j{j}", name=f"msk{g}j{j}") for j in range(GB)]
                   for g in range(NG)]
            drows_sb = [sb.tile([GB, W], f32, tag=f"drows{g}", name=f"drows{g}") for g in range(NG)]
            centg = [sb.tile([GB, 3], f32, tag=f"centg{g}", name=f"centg{g}") for g in range(NG)]
            crow = [sb.tile([1, GB * 3], f32, tag=f"crow{g}", name=f"crow{g}") for g in range(NG)]
            outP = [sb.tile([GB, NPOINT], i32, tag=f"outP{g}", name=f"outP{g}") for g in range(NG)]
            glocP = [sb.tile([GB, NPOINT], f32, tag=f"glocP{g}", name=f"glocP{g}") for g in range(NG)]
            flagsP = [sb.tile([GB, NPOINT], f32, tag=f"flagsP{g}", name=f"flagsP{g}") for g in range(NG)]
            rowcnt = [sb.tile([GB, 1], f32, tag=f"rowcnt{g}", name=f"rowcnt{g}") for g in range(NG)]
            hist = [[sb.tile([GB, NPOINT], f32, tag=f"h{c}g{g}", name=f"h{c}g{g}") for c in range(3)]
                    for g in range(NG)]

            # PSUM: one small-stuff bank + one drows bank per group
            bankA = [ps.tile([P, 512], f32, tag=f"bankA{g}", name=f"bankA{g}") for g in range(NG)]
            drows_ps = [ps.tile([GB, W], f32, tag=f"drps{g}", name=f"drps{g}") for g in range(NG)]

            def emit_bias_chain(g):
                # goff[g] [GB,1] u32 -> centroid xyz -> bias[g] [128, GB*3]
                nc.gpsimd.indirect_dma_start(
                    out=centg[g][:],
                    out_offset=None,
                    in_=tbl[:],
                    in_offset=bass.IndirectOffsetOnAxis(ap=goff[g][:], axis=0),
                )
                nc.sync.dma_start(
                    crow[g][:].rearrange("o (j c) -> o j c", j=GB), centg[g][:]
                )
                bias_ps = bankA[g][:, 16 : 16 + GB * 3]
                nc.tensor.matmul(bias_ps, ones1[:], crow[g][:], start=True, stop=True)
                nc.scalar.copy(bias[g][:], bias_ps)

            for g in range(NG):
                nc.vector.memset(D[g][:], 1e10)
                nc.vector.memset(glocP[g][:], 0)
                nc.vector.memset(flagsP[g][:], 0)
                for c in range(3):
                    nc.vector.memset(hist[g][c][:], 1e15)
                # initial centroid = point 0 of each batch
                nc.vector.tensor_copy(goff[g][:], binit[:, g : g + 1])
                emit_bias_chain(g)
                for c in range(3):
                    nc.vector.tensor_copy(hist[g][c][:, 0:1], centg[g][:, c : c + 1])

            # ---- main loop ----
            def emit_iter(k):
                # k: python int or ScalarValue expr; only used for the
                # outP column offset
                for g in range(NG):
                    # squares: sq_c = Square(-C + c_c) ; bias AP per batch col
                    for c in range(3):
                        for j in range(GB):
                            nc.scalar.activation(
                                SQ[g][c][:, j * W : (j + 1) * W],
                                C[g][c][:, j * W : (j + 1) * W],
                                Act.Square,
                                bias=bias[g][:, j * 3 + c : j * 3 + c + 1],
                                scale=-1.0,
                            )
                    nc.vector.tensor_tensor(
                        out=S1[g][:], in0=SQ[g][0][:], in1=SQ[g][1][:], op=Alu.add
                    )
                    nc.vector.tensor_tensor(
                        out=S2[g][:], in0=S1[g][:], in1=SQ[g][2][:], op=Alu.add
                    )
                    nc.vector.tensor_tensor(
                        out=D[g][:], in0=D[g][:], in1=S2[g][:], op=Alu.min
                    )
                    nc.vector.tensor_reduce(
                        out=rowmax[g][:],
                        in_=D[g][:].rearrange("p (j w) -> p j w", j=GB),
                        axis=Ax.X,
                        op=Alu.max,
                    )
                    # global max + first-index argmax via exact arithmetic:
                    # cand = idx + BIG*(1 - (val == gmax)); argfirst = min(cand)
                    rmT_ps = bankA[g][0:GB, 32 : 32 + P]
                    nc.tensor.transpose(rmT_ps, rowmax[g][:], ident[:])
                    nc.scalar.copy(rmT[g][:], rmT_ps)
                    nc.vector.tensor_reduce(
                        out=gmaxF[g][:], in_=rmT[g][:], axis=Ax.X, op=Alu.max
                    )
                    nc.vector.scalar_tensor_tensor(
                        out=eqn[g][:],
                        in0=rmT[g][:],
                        scalar=gmaxF[g][:],
                        in1=negbig[0:GB, 0:1].to_broadcast([GB, P]),
                        op0=Alu.is_equal,
                        op1=Alu.mult,
                    )
                    nc.vector.tensor_tensor(
                        out=cand[g][:], in0=eqn[g][:], in1=piotaBig[0:GB, :], op=Alu.add
                    )
                    nc.vector.tensor_reduce(
                        out=pF[g][:], in_=cand[g][:], axis=Ax.X, op=Alu.min
                    )
                    # winner-row mask
                    pT_ps = bankA[g][0:1, 0:GB]
                    nc.tensor.transpose(pT_ps, pF[g][:], ident[0:GB, 0:GB])
                    nc.scalar.copy(prow[g][:], pT_ps)
                    pbc_ps = bankA[g][:, 4 : 4 + GB]
                    nc.tensor.matmul(pbc_ps, ones1[:], prow[g][:], start=True, stop=True)
                    nc.scalar.copy(pbc_sb[g][:], pbc_ps)
                    nc.vector.tensor_tensor(
                        out=rowmask[g][:], in0=piota[:, 0:GB], in1=pbc_sb[g][:],
                        op=Alu.is_equal,
                    )
                    for j in range(GB):
                        nc.vector.tensor_tensor(
                            out=msk[g][j][:],
                            in0=rowmask[g][:],
                            in1=colsel[:, j * GB : (j + 1) * GB],
                            op=Alu.mult,
                        )
                    # extract winner D rows (one-hot matmuls, exact)
                    for j in range(GB):
                        nc.tensor.matmul(
                            drows_ps[g][:],
                            msk[g][j][:],
                            D[g][:, j * W : (j + 1) * W],
                            start=(j == 0),
                            stop=(j == GB - 1),
                        )
                    nc.scalar.copy(drows_sb[g][:], drows_ps[g][:])
                    # first column equal to gmax within the winner row
                    nc.vector.scalar_tensor_tensor(
                        out=eqn2[g][:],
                        in0=drows_sb[g][:],
                        scalar=gmaxF[g][:],
                        in1=negbig[0:GB, 0:1].to_broadcast([GB, W]),
                        op0=Alu.is_equal,
                        op1=Alu.mult,
                    )
                    nc.vector.tensor_tensor(
                        out=cand2[g][:], in0=eqn2[g][:], in1=qiotaBig[0:GB, :],
                        op=Alu.add,
                    )
                    nc.vector.tensor_reduce(
                        out=qF[g][:], in_=cand2[g][:], axis=Ax.X, op=Alu.min
                    )
                    # gloc = p*512 + q (the output index), gflat = gloc + b*65536
                    nc.vector.scalar_tensor_tensor(
                        out=gloc[g][:],
                        in0=pF[g][:],
                        scalar=float(W),
                        in1=qF[g][:],
                        op0=Alu.mult,
                        op1=Alu.add,
                    )
                    nc.vector.tensor_tensor(
                        out=gflat[g][:], in0=gloc[g][:], in1=bbasef[:, g : g + 1],
                        op=Alu.add,
                    )
                    # output indices for step k+1 (partition-major)
                    nc.vector.tensor_copy(
                        glocP[g][:, bass.ds(k + 1, 1)], gloc[g][:]
                    )
                    # cross-row tie flag: sum(eqn) = -BIG * (#rows at max)
                    nc.vector.tensor_reduce(
                        out=rowcnt[g][:], in_=eqn[g][:], axis=Ax.X, op=Alu.add
                    )
                    nc.vector.tensor_copy(
                        flagsP[g][:, bass.ds(k + 1, 1)], rowcnt[g][:]
                    )
                    # gather next centroid xyz -> bias (+ record history)
                    nc.vector.tensor_copy(goff[g][:], gflat[g][:])
                    emit_bias_chain(g)
                    for c in range(3):
                        nc.vector.tensor_copy(
                            hist[g][c][:, bass.ds(k + 1, 1)], centg[g][:, c : c + 1]
                        )

            UNROLL = 4
            n_main = (niter // UNROLL) * UNROLL
            if n_main > 0:
                with tc.For_i(0, n_main, UNROLL) as iv:
                    for u in range(UNROLL):
                        emit_iter(iv + u)
            for k in range(n_main, niter):
                emit_iter(k)

            # ---- post-loop tie resolution ----
            # Rare case: two points' plain-f32 distances tie bit-exactly at
            # the global max, but the XLA-CPU reference (which computes
            # dist = fma(dz,dz, fma(dx,dx, dy*dy)) with fused single-rounding
            # multiply-adds) orders them strictly. Our plain kernel picks the
            # smaller index and (verified) picks the partner on the next step;
            # the reference may emit them in the other order. Here we find
            # flagged steps and recompute the two candidates' exact-FMA
            # distances via TwoProduct/TwoSum emulation, swapping the pair in
            # the output when the reference order is the reverse. Branchless:
            # empty slots resolve to no-op updates.
            NSLOT = 3
            MASKC = 0xFFFFF000

            _sc = {nm: sb.tile([GB, NPOINT], f32, tag=nm, name=nm)
                   for nm in ("sc1", "sc2", "sc3", "sc4", "sc5", "sc6", "sc7",
                               "kc", "oh1", "oh2", "facc")}
            sc1 = [_sc["sc1"]] * NG
            sc2 = [_sc["sc2"]] * NG
            sc3 = [_sc["sc3"]] * NG
            sc4 = [_sc["sc4"]] * NG
            sc5 = [_sc["sc5"]] * NG
            sc6 = [_sc["sc6"]] * NG
            sc7 = [_sc["sc7"]] * NG
            kc = [_sc["kc"]] * NG
            oh1 = [_sc["oh1"]] * NG
            oh2 = [_sc["oh2"]] * NG
            facc = [_sc["facc"]] * NG
            ksF = [sb.tile([GB, 1], f32, tag=f"ksF{g}", name=f"ksF{g}") for g in range(NG)]
            i1F = [sb.tile([GB, 1], f32, tag=f"i1F{g}", name=f"i1F{g}") for g in range(NG)]
            i2F = [sb.tile([GB, 1], f32, tag=f"i2F{g}", name=f"i2F{g}") for g in range(NG)]
            dv = [[sb.tile([GB, 1], f32, tag=f"dv{g}_{t}", name=f"dv{g}_{t}") for t in range(2)]
                  for g in range(NG)]
            swp = [sb.tile([GB, 1], f32, tag=f"swp{g}", name=f"swp{g}") for g in range(NG)]
            cgath = [sb.tile([GB, 3], f32, tag=f"cgath{g}", name=f"cgath{g}") for g in range(NG)]
            goff2 = [sb.tile([GB, 1], u32, tag=f"goff2{g}", name=f"goff2{g}") for g in range(NG)]

            def fma_step(g, dxt, acc_in, acc_out):
                # acc_out = fl(dxt*dxt + acc_in), single rounding (TwoProduct
                # + TwoSum; all intermediate ops exact per Dekker/Knuth)
                Ph, am, al, A, Bp, Cc = sc2[g], sc3[g], sc4[g], sc5[g], sc6[g], sc7[g]
                nc.vector.tensor_tensor(out=Ph[:], in0=dxt[:], in1=dxt[:], op=Alu.mult)
                nc.vector.tensor_scalar(
                    out=am[:].bitcast(u32), in0=dxt[:].bitcast(u32),
                    scalar1=MASKC, scalar2=None, op0=Alu.bitwise_and,
                )
                nc.vector.tensor_tensor(out=al[:], in0=dxt[:], in1=am[:], op=Alu.subtract)
                nc.vector.tensor_tensor(out=A[:], in0=am[:], in1=am[:], op=Alu.mult)
                nc.vector.tensor_tensor(out=Bp[:], in0=am[:], in1=al[:], op=Alu.mult)
                nc.vector.tensor_tensor(out=Cc[:], in0=al[:], in1=al[:], op=Alu.mult)
                nc.vector.tensor_tensor(out=A[:], in0=A[:], in1=Ph[:], op=Alu.subtract)
                nc.vector.scalar_tensor_tensor(
                    out=Bp[:], in0=Bp[:], scalar=2.0, in1=A[:],
                    op0=Alu.mult, op1=Alu.add,
                )
                nc.vector.tensor_tensor(out=Cc[:], in0=Bp[:], in1=Cc[:], op=Alu.add)
                # Cc = Pl (exact product error). TwoSum(Ph, acc_in):
                s, bv, tb = am, al, A  # reuse scratch
                nc.vector.tensor_tensor(out=s[:], in0=Ph[:], in1=acc_in[:], op=Alu.add)
                nc.vector.tensor_tensor(out=bv[:], in0=s[:], in1=Ph[:], op=Alu.subtract)
                nc.vector.tensor_tensor(out=tb[:], in0=s[:], in1=bv[:], op=Alu.subtract)
                nc.vector.tensor_tensor(out=tb[:], in0=Ph[:], in1=tb[:], op=Alu.subtract)
                nc.vector.tensor_tensor(out=bv[:], in0=acc_in[:], in1=bv[:], op=Alu.subtract)
                nc.vector.tensor_tensor(out=tb[:], in0=tb[:], in1=bv[:], op=Alu.add)
                nc.vector.tensor_tensor(out=tb[:], in0=tb[:], in1=Cc[:], op=Alu.add)
                nc.vector.tensor_tensor(out=acc_out[:], in0=s[:], in1=tb[:], op=Alu.add)

            def fma_exact_min_d(g, cg, dst):
                # dst[GB,1] = min over valid centroids m of
                #   fma(dz,dz, fma(dx,dx, dy*dy)) vs point coords cg[GB,3]
                dt, acc = sc1[g], facc[g]
                # t0 = dy*dy with dy = fl(y - cy)
                nc.vector.tensor_scalar(
                    out=dt[:], in0=hist[g][1][:], scalar1=cg[:, 1:2],
                    scalar2=-1.0, op0=Alu.subtract, op1=Alu.mult,
                )
                nc.vector.tensor_tensor(out=acc[:], in0=dt[:], in1=dt[:], op=Alu.mult)
                # += dx*dx (fused)
                nc.vector.tensor_scalar(
                    out=dt[:], in0=hist[g][0][:], scalar1=cg[:, 0:1],
                    scalar2=-1.0, op0=Alu.subtract, op1=Alu.mult,
                )
                fma_step(g, dt, acc, acc)
                # += dz*dz (fused)
                nc.vector.tensor_scalar(
                    out=dt[:], in0=hist[g][2][:], scalar1=cg[:, 2:3],
                    scalar2=-1.0, op0=Alu.subtract, op1=Alu.mult,
                )
                fma_step(g, dt, acc, acc)
                # mask out centroids m >= ks, then min
                nc.vector.tensor_scalar(
                    out=dt[:], in0=iota1k[0:GB, :], scalar1=ksF[g][:, 0:1],
                    scalar2=1e30, op0=Alu.is_ge, op1=Alu.mult,
                )
                nc.vector.tensor_tensor(out=acc[:], in0=acc[:], in1=dt[:], op=Alu.add)
                nc.vector.tensor_reduce(out=dst[:], in_=acc[:], axis=Ax.X, op=Alu.min)

            for g in range(NG):
                for slot in range(NSLOT):
                    if slot == 0:
                        # kcand = iota + BIG - BIG*flagged
                        nc.vector.tensor_scalar(
                            out=sc1[g][:], in0=flagsP[g][:], scalar1=-1.5 * 1048576.0,
                            scalar2=None, op0=Alu.is_le,
                        )
                        nc.vector.scalar_tensor_tensor(
                            out=kc[g][:], in0=sc1[g][:], scalar=-1048576.0,
                            in1=iotaBig1k[0:GB, :], op0=Alu.mult, op1=Alu.add,
                        )
                    else:
                        # mask out the previous slot's step
                        nc.vector.tensor_scalar(
                            out=sc1[g][:], in0=iota1k[0:GB, :], scalar1=ksF[g][:, 0:1],
                            scalar2=0.0, op0=Alu.subtract, op1=Alu.is_equal,
                        )
                        nc.vector.scalar_tensor_tensor(
                            out=kc[g][:], in0=sc1[g][:], scalar=1048576.0,
                            in1=kc[g][:], op0=Alu.mult, op1=Alu.add,
                        )
                    nc.vector.tensor_reduce(
                        out=ksF[g][:], in_=kc[g][:], axis=Ax.X, op=Alu.min
                    )
                    # one-hots for step ks and ks+1
                    nc.vector.tensor_scalar(
                        out=oh1[g][:], in0=iota1k[0:GB, :], scalar1=ksF[g][:, 0:1],
                        scalar2=0.0, op0=Alu.subtract, op1=Alu.is_equal,
                    )
                    nc.vector.tensor_scalar(
                        out=oh2[g][:], in0=iota1k[0:GB, :], scalar1=ksF[g][:, 0:1],
                        scalar2=1.0, op0=Alu.subtract, op1=Alu.is_equal,
                    )
                    nc.vector.tensor_tensor(
                        out=sc1[g][:], in0=glocP[g][:], in1=oh1[g][:], op=Alu.mult
                    )
                    nc.vector.tensor_reduce(
                        out=i1F[g][:], in_=sc1[g][:], axis=Ax.X, op=Alu.add
                    )
                    nc.vector.tensor_tensor(
                        out=sc1[g][:], in0=glocP[g][:], in1=oh2[g][:], op=Alu.mult
                    )
                    nc.vector.tensor_reduce(
                        out=i2F[g][:], in_=sc1[g][:], axis=Ax.X, op=Alu.add
                    )
                    # exact-FMA min distances of both candidates
                    for t, iF in ((0, i1F), (1, i2F)):
                        nc.vector.tensor_tensor(
                            out=swp[g][:], in0=iF[g][:], in1=bbasef[:, g : g + 1],
                            op=Alu.add,
                        )
                        nc.vector.tensor_copy(goff2[g][:], swp[g][:])
                        nc.gpsimd.indirect_dma_start(
                            out=cgath[g][:],
                            out_offset=None,
                            in_=tbl[:],
                            in_offset=bass.IndirectOffsetOnAxis(ap=goff2[g][:], axis=0),
                        )
                        fma_exact_min_d(g, cgath[g], dv[g][t])
                    # swap iff reference (FMA) order picks i2 first
                    nc.vector.tensor_tensor(
                        out=swp[g][:], in0=dv[g][1][:], in1=dv[g][0][:], op=Alu.is_gt
                    )
                    nc.vector.tensor_tensor(
                        out=i2F[g][:], in0=i2F[g][:], in1=i1F[g][:], op=Alu.subtract
                    )
                    nc.vector.tensor_tensor(
                        out=i2F[g][:], in0=i2F[g][:], in1=swp[g][:], op=Alu.mult
                    )
                    nc.vector.tensor_tensor(
                        out=oh1[g][:], in0=oh1[g][:], in1=oh2[g][:], op=Alu.subtract
                    )
                    nc.vector.tensor_tensor(
                        out=oh1[g][:],
                        in0=oh1[g][:],
                        in1=i2F[g][:, 0:1].to_broadcast([GB, NPOINT]),
                        op=Alu.mult,
                    )
                    nc.vector.tensor_tensor(
                        out=glocP[g][:], in0=glocP[g][:], in1=oh1[g][:], op=Alu.add
                    )

            for g in range(NG):
                nc.vector.tensor_copy(outP[g][:], glocP[g][:])
                nc.sync.dma_start(out[g * GB : (g + 1) * GB, :], outP[g][:])

    nc.finalize()
    return nc


def _consts():
    BIG = 1048576.0  # 2**20: idx + BIG stays exactly representable in f32
    ident = np.eye(P, dtype=np.float32)
    ones1 = np.ones((1, P), np.float32)
    piota = np.tile(np.arange(P, dtype=np.float32)[:, None], (1, GB))
    colsel = np.zeros((P, GB * GB), np.float32)
    for j in range(GB):
        colsel[:, j * GB + j] = 1.0
    bbasef = np.zeros((GB, NG), np.float32)
    binit = np.zeros((GB, NG), np.uint32)
    for g in range(NG):
        for j in range(GB):
            bbasef[j, g] = float((g * GB + j) * N)
            binit[j, g] = (g * GB + j) * N
    piotaBig = np.tile(np.arange(P, dtype=np.float32)[None, :] + BIG, (GB, 1))
    qiotaBig = np.tile(np.arange(W, dtype=np.float32)[None, :] + BIG, (GB, 1))
    negbig = np.full((GB, 1), -BIG, np.float32)
    iota1k = np.tile(np.arange(NPOINT, dtype=np.float32)[None, :], (GB, 1))
    iotaBig1k = iota1k + BIG
    return {
        "iota1k": iota1k,
        "iotaBig1k": iotaBig1k,
        "ident": ident,
        "ones1": ones1,
        "piota": piota,
        "colsel": colsel,
        "bbasef": bbasef,
        "binit": binit,
        "piotaBig": piotaBig,
        "qiotaBig": qiotaBig,
        "negbig": negbig,
    }


LAST_RUN_INFO = {}


def kernel(x: np.ndarray, niter: int = NITER, trace: bool = False) -> np.ndarray:
    x = np.asarray(x)
    assert x.shape == (32, 65536, 6) and x.dtype == np.float32
    key = ("nc", niter)
    if key not in _CACHED:
        _CACHED[key] = build_nc(niter)
    nc = _CACHED[key]
    consts = _consts()
    in_maps = []
    for core in range(8):
        xs = x[core * NB : (core + 1) * NB]  # [4, 65536, 6]
        xr = np.ascontiguousarray(xs.reshape(NB, P, W * 6))
        in_maps.append({"x": xr, **consts})
    res = run_bass_kernel_spmd(nc, in_maps, list(range(8)), trace=trace)
    LAST_RUN_INFO["exec_time_ns"] = getattr(res, "exec_time_ns", None)
    LAST_RUN_INFO["profile_json"] = getattr(res, "profile_json", None)
    outs = np.zeros((32, NPOINT), np.int32)
    for core in range(8):
        o = np.asarray(res.results[core]["out"])  # [NB, NPOINT]
        outs[core * NB : (core + 1) * NB] = o
    return outs[:, : niter + 1] if niter != NITER else outs


if __name__ == "__main__":
    rng = np.random.default_rng(0)
    x = rng.standard_normal((32, 65536, 6)).astype(np.float32)
    out = kernel(x)
    print(out.shape, out.dtype, out[:2, :8])
